# revision 19
# baseline (speedup 1.0000x reference)
"""Trainium2 Bass kernel for CapsuleLayer (nn_CapsuleLayer_45552423142009).

Computes, for x[B,768]:
  u = squash(x @ Wp + bp)            # [B, 8, 16]  (squash over last dim)
  u_hat[b,p,c,:] = u[b,p,:] @ W[p,c] # [B, 8, 5, 16]
  3 iterations of dynamic routing -> v [B, 5, 16]

Strategy: pure data-parallel over 8 NeuronCores (batch sharded 16384/core).
On-chip layout is "transposed": features on partitions, batch on the free
dim (512-wide tiles).  Key differences vs the v1 kernel:
  - x is transposed + cast to fp16 on the host, so the device does no
    PE transposes of x and reads half the HBM bytes.
  - all matmul operands are fp16 (1 cycle/row on the PE vs ~2 for f32r).
  - squash factors via exp/ln only (f = sqrt(q)/(1+q) = exp(.5*ln q -
    ln(1+q))), so the ACT engine stays on one table set (no ~2.7us
    ACT_TABLE_LOAD thrash), and softmax recip uses the fast DVE approx.
  - s/v are packed [80=(c,j), b] via M=80 accumulating selector matmuls:
    one vsq / g-broadcast / v-multiply per iteration instead of five.
  - routing logits accumulate directly in one pinned PSUM bank across
    iterations (PE start=False accumulation), read in place by Exp.
"""

import sys
import numpy as np

sys.path.insert(0, "/opt/trn_rl_repo")

from concourse import bass, bacc, mybir  # noqa: E402
from concourse import tile  # noqa: E402
from concourse.bass_utils import run_bass_kernel_spmd  # noqa: E402
from concourse.alu_op_type import AluOpType  # noqa: E402

F32 = mybir.dt.float32
F16 = mybir.dt.float16
AF = mybir.ActivationFunctionType

B = 131072
D = 768
P = 8
PD = 16
C = 5
CD = 16
NCORES = 8
BC = B // NCORES          # 16384 batch rows per core
NB = 512                  # batch columns per tile
NT = BC // NB             # 32 tiles

# fp16 constant blob column offsets
OWP = 0                   # [128, 768]   mm1 stationary (6 chunks of 128)
OWBD = 768                # [128, 640]   u_hat stationary, per class [128,128]
OS0 = 1408                # [128, 80]    itr0 s80 stationary (0.2*W, all classes)
OWFL = 1808               # [128, 400]   itr>0 s80 stationary (W per class)
OBSEL = 2208              # [40, 640]    cn -> (p,i) broadcast, per class
OASEL = 2848              # [128, 200]   agreement contraction, per class
OVBC = 3048               # [80, 640]    v80 -> (p,j) broadcast, per class
OSSB = 3688               # [128, 128]   fused: qb[(p,j)] = sum_j' usq[(p,j')]
OJGB = 3816               # [80, 80]     fused: qv[(c,j)] = sum_j' ssq[(c,j')]
OCC = 3896                # [40, 40]     fused: den[(c,p)] = sum_c' e[(c',p)]
OID80 = 3944              # [80, 80]     identity (v transposes)
CW = 4024


def build_consts(Wp, bp, W):
    """Host-side packing of all selectors + weights into one fp16 blob
    plus the fp32 bias column."""
    Wp = np.asarray(Wp, np.float32)
    bp = np.asarray(bp, np.float32)
    W = np.asarray(W, np.float32)

    cst = np.zeros((128, CW), np.float32)

    # mm1 stationary: wp[(d_sub), k*128 + (p,o)] = Wp[d, p, o] with d = k*128+d_sub
    wp_flat = Wp.transpose(1, 0, 2).reshape(D, 128)            # [d, (p,o)]
    cst[:, OWP:OWP + 768] = wp_flat.reshape(6, 128, 128).transpose(1, 0, 2).reshape(128, 768)

    # u_hat stationary (block-diag over p): wbd[(p,i), c*128+(p,j)] = W[p,c,i,j]
    for p in range(P):
        for c in range(C):
            cst[p * 16:(p + 1) * 16, OWBD + c * 128 + p * 16:OWBD + c * 128 + (p + 1) * 16] = W[p, c]

    # itr0 s in one matmul: s0[(c,j)] = .2 sum_p u_p @ W_pc
    # os0[(p,i), c*16+j] = .2 * W[p,c,i,j]
    for c in range(C):
        for p in range(P):
            cst[p * 16:(p + 1) * 16, OS0 + c * 16:OS0 + (c + 1) * 16] += 0.2 * W[p, c]

    # itr>0 s: wfl[(p,i), c*80 + (c', j)] = W[p,c,i,j] * d_c'c
    for c in range(C):
        for p in range(P):
            cst[p * 16:(p + 1) * 16, OWFL + c * 80 + c * 16:OWFL + c * 80 + (c + 1) * 16] = W[p, c]

    # bsel[(c'p'), c*128 + (p,i)] = d_c'c d_p'p
    for c in range(C):
        for p in range(P):
            cst[c * 8 + p, OBSEL + c * 128 + p * 16:OBSEL + c * 128 + (p + 1) * 16] = 1.0

    # asel[(p,j), c*40 + (c',p')] = d_pp' d_c'c
    for c in range(C):
        for p in range(P):
            for j in range(CD):
                cst[p * 16 + j, OASEL + c * 40 + c * 8 + p] = 1.0

    # vbc[(c'',j'), c*128 + (p,j)] = d_c''c d_j'j
    for c in range(C):
        for p in range(P):
            for j in range(CD):
                cst[c * 16 + j, OVBC + c * 128 + p * 16 + j] = 1.0

    # fused sum-then-broadcast within 16-row groups: block all-ones
    for p in range(P):
        cst[p * 16:(p + 1) * 16, OSSB + p * 16:OSSB + (p + 1) * 16] = 1.0
    for c in range(C):
        cst[c * 16:(c + 1) * 16, OJGB + c * 16:OJGB + (c + 1) * 16] = 1.0

    # fused softmax denominator: occ[(c',p'), (c,p)] = d_p'p
    for c2 in range(C):
        for c in range(C):
            for p in range(P):
                cst[c2 * 8 + p, OCC + c * 8 + p] = 1.0

    cst[:80, OID80:OID80 + 80] = np.eye(80)

    bp_h = np.ascontiguousarray(bp.reshape(128, 1), dtype=np.float32)
    return np.ascontiguousarray(cst.astype(np.float16)), bp_h


def prep_x(x_core):
    """[bc, 768] fp32 -> tile-image [nt*128, 3072] fp16 with
    img[t*128+p, k*512+c] = x[t*512+c, k*128+p] (pre-transposed)."""
    bc = x_core.shape[0]
    nt = bc // NB
    xi = x_core.reshape(nt, NB, 6, 128).transpose(0, 3, 2, 1)  # [t, p, k, c]
    return np.ascontiguousarray(xi.reshape(nt * 128, 6 * NB), dtype=np.float16)


def build_nc(nt: int = NT) -> bass.Bass:
    bc = nt * NB
    nc = bacc.Bacc(None)

    x_d = nc.declare_dram_parameter("xt", [nt * 128, 6 * NB], F16, isOutput=False)
    cb_d = nc.declare_dram_parameter("cstb", [128, CW], F16, isOutput=False)
    cf_d = nc.declare_dram_parameter("cstf", [128, 1], F32, isOutput=False)
    v_d = nc.declare_dram_parameter("vout", [nt * 80, NB], F16, isOutput=True)

    with tile.TileContext(nc) as tc, nc.allow_low_precision(reason="fp16 compute"):
        with (
            tc.sbuf_pool(name="const", bufs=1) as cpool,
            tc.sbuf_pool(name="xin", bufs=4) as xpool,
            tc.sbuf_pool(name="mid", bufs=3) as mpool,
            tc.sbuf_pool(name="uh", bufs=3) as uhpool,
            tc.sbuf_pool(name="rt", bufs=3) as rtpool,
            tc.sbuf_pool(name="sm", bufs=4) as smpool,
            tc.sbuf_pool(name="vo", bufs=2) as vopool,
            tc.psum_pool(name="pfront", bufs=2) as pfr,
            tc.psum_pool(name="pbcast", bufs=2) as pbc,
            tc.psum_pool(name="ps80", bufs=2) as ps80p,
            tc.psum_pool(name="psm", bufs=2) as psmp,
        ):
            # ---- pin the one ACT table set holding Ln+Exp+Square+Copy+
            # Identity, so the first-fit table chooser never thrashes ----
            nc.scalar.add_instruction(mybir.InstLoadActFuncSet(
                name=nc.get_next_instruction_name(), ins=[], outs=[],
                act_func_set_id=6))  # natural_log_exp_and_others

            # ---- constants: one DMA each, staged through DVE ----
            cst0 = cpool.tile([128, CW], F16)
            nc.sync.dma_start(out=cst0[:], in_=cb_d[:])
            cst = cpool.tile([128, CW], F16)
            nc.vector.tensor_copy(cst[:], cst0[:])
            bp0 = cpool.tile([128, 1], F32)
            nc.sync.dma_start(out=bp0[:], in_=cf_d[:])
            bp_sb = cpool.tile([128, 1], F32)
            nc.vector.tensor_copy(bp_sb[:], bp0[:])

            def emit_front(it, outs):
                xts = xpool.tile([128, 6 * NB], F16, tag="xin")
                nc.sync.dma_start(out=xts[:], in_=x_d[it * 128:(it + 1) * 128, :])
                yield

                # ---- mm1: u_pre[(p,o), b] = Wp^T x^T + bp ----
                pm = pfr.tile([128, NB], F32, tag="fr")
                for k in range(6):
                    nc.tensor.matmul(
                        pm[:], cst[:, OWP + k * 128:OWP + (k + 1) * 128],
                        xts[:, k * NB:(k + 1) * NB],
                        start=(k == 0), stop=(k == 5))
                    yield
                u_pre = mpool.tile([128, NB], F16, tag="upre")
                nc.scalar.activation(u_pre[:], pm[:], AF.Identity,
                                     bias=bp_sb[:], scale=1.0)
                yield
                usq = mpool.tile([128, NB], F16, tag="usq")
                nc.scalar.activation(usq[:], pm[:], AF.Square,
                                     bias=bp_sb[:], scale=1.0)
                yield

                # ---- squash factor f = exp(.5 ln q - ln(1+q)), computed
                # directly at (p,j) width via the fused sum+bcast selector ----
                pq = pfr.tile([128, NB], F32, tag="fr")
                nc.tensor.matmul(pq[:], cst[:, OSSB:OSSB + 128], usq[:],
                                 start=True, stop=True)
                yield
                lnq = smpool.tile([128, NB], F32, tag="lna")
                nc.scalar.activation(lnq[:], pq[:], AF.Ln)
                yield
                l1q = smpool.tile([128, NB], F32, tag="lnb")
                nc.scalar.activation(l1q[:], pq[:], AF.Ln, bias=1.0)
                yield
                fz = smpool.tile([128, NB], F16, tag="ff")
                nc.vector.scalar_tensor_tensor(
                    fz[:], lnq[:], 0.5, l1q[:],
                    op0=AluOpType.mult, op1=AluOpType.subtract)
                yield
                fb = smpool.tile([128, NB], F16, tag="fb")
                nc.scalar.activation(fb[:], fz[:], AF.Exp)
                yield
                u = mpool.tile([128, NB], F16, tag="uu")
                nc.vector.tensor_mul(u[:], u_pre[:], fb[:])
                yield

                # ---- u_hat per class ----
                uh = []
                for c in range(C):
                    puh = pfr.tile([128, NB], F32, tag="fr")
                    nc.tensor.matmul(
                        puh[:], cst[:, OWBD + c * 128:OWBD + (c + 1) * 128],
                        u[:], start=True, stop=True)
                    yield
                    uhc = uhpool.tile([128, NB], F16, tag=f"uh{c}")
                    if c in (0, 1, 3):
                        nc.scalar.copy(uhc[:], puh[:])  # ACT/DVE balance
                    else:
                        nc.vector.tensor_copy(uhc[:], puh[:])
                    uh.append(uhc)
                    yield
                ps0 = ps80p.tile([80, NB], F32, tag="ps")
                nc.tensor.matmul(ps0[:], cst[:, OS0:OS0 + 80], u[:],
                                 start=True, stop=True)
                outs[it] = (u, uh, ps0)

            def emit_routing(it, u, uh, ps0):
                lg_sb = None

                for itr in range(3):
                    yield
                    if itr == 0:
                        ps = ps0
                    else:
                        # softmax over classes of logits [ (c,p), b ]
                        e = rtpool.tile([40, NB], F16, tag="ee")
                        nc.scalar.activation(e[:], lg_sb[:], AF.Exp)
                        yield
                        pden = psmp.tile([40, NB], F32, tag="sm")
                        nc.tensor.matmul(pden[:], cst[:40, OCC:OCC + 40],
                                         e[:], start=True, stop=True)
                        yield
                        rdf = smpool.tile([40, NB], F32, tag="rdf")
                        nc.vector.reciprocal_approx_fast(out=rdf[:], in_=pden[:])
                        yield
                        cn = rtpool.tile([40, NB], F16, tag="cn")
                        nc.vector.tensor_mul(cn[:], e[:], rdf[:])
                        yield

                        ps = ps80p.tile([80, NB], F32, tag="ps")
                        for c in range(C):
                            pcb = pbc.tile([128, NB], F32, tag="bc")
                            nc.tensor.matmul(
                                pcb[:], cst[:40, OBSEL + c * 128:OBSEL + (c + 1) * 128],
                                cn[:], start=True, stop=True)
                            yield
                            tcm = rtpool.tile([128, NB], F16, tag=f"t{c}")
                            nc.vector.tensor_mul(tcm[:], u[:], pcb[:])
                            yield
                            nc.tensor.matmul(
                                ps[:], cst[:, OWFL + c * 80:OWFL + (c + 1) * 80],
                                tcm[:], start=(c == 0), stop=(c == 4))
                            yield

                    # ---- g = squash factor of s, computed directly at
                    # (c,j) width via the fused sum+bcast selector ----
                    ssq = rtpool.tile([80, NB], F16, tag="ssq")
                    nc.scalar.activation(ssq[:], ps[:], AF.Square)
                    yield
                    pvq = psmp.tile([80, NB], F32, tag="sm")
                    nc.tensor.matmul(pvq[:], cst[:80, OJGB:OJGB + 80],
                                     ssq[:], start=True, stop=True)
                    yield
                    lnv = smpool.tile([80, NB], F32, tag="lnc")
                    nc.scalar.activation(lnv[:], pvq[:], AF.Ln)
                    yield
                    l1v = smpool.tile([80, NB], F32, tag="lnd")
                    nc.scalar.activation(l1v[:], pvq[:], AF.Ln, bias=1.0)
                    yield
                    zv = smpool.tile([80, NB], F16, tag="zv")
                    nc.vector.scalar_tensor_tensor(
                        zv[:], lnv[:], 0.5, l1v[:],
                        op0=AluOpType.mult, op1=AluOpType.subtract)
                    yield
                    gb = rtpool.tile([80, NB], F16, tag="gb")
                    nc.scalar.activation(gb[:], zv[:], AF.Exp)
                    yield
                    v80 = rtpool.tile([80, NB], F16, tag="v80")
                    nc.vector.tensor_mul(v80[:], gb[:], ps[:])
                    yield

                    if itr < 2:
                        # logits += sum_j uh*v
                        pat = psmp.tile([40, NB], F32, tag="sm")
                        for c in range(C):
                            pvb = pbc.tile([128, NB], F32, tag="bc")
                            nc.tensor.matmul(
                                pvb[:], cst[:80, OVBC + c * 128:OVBC + (c + 1) * 128],
                                v80[:], start=True, stop=True)
                            yield
                            pr = rtpool.tile([128, NB], F16, tag=f"pr{c}")
                            nc.vector.tensor_mul(pr[:], uh[c][:], pvb[:])
                            yield
                            nc.tensor.matmul(
                                pat[:], cst[:, OASEL + c * 40:OASEL + (c + 1) * 40],
                                pr[:], start=(c == 0), stop=(c == 4))
                            yield
                        if itr == 0:
                            lg_sb = rtpool.tile([40, NB], F32, tag="lg")
                            nc.scalar.copy(lg_sb[:], pat[:])
                            yield
                        else:
                            lg2 = rtpool.tile([40, NB], F32, tag="lg2")
                            nc.vector.tensor_add(lg2[:], lg_sb[:], pat[:])
                            lg_sb = lg2
                            yield
                    else:
                        # store v80 [(c,j), b] directly; host transposes
                        nc.sync.dma_start(
                            out=v_d[it * 80:(it + 1) * 80, :], in_=v80[:])

            def drain(g, n):
                if g is None:
                    return False
                for _ in range(n):
                    try:
                        next(g)
                    except StopIteration:
                        return False
                return True

            outs = {}
            fg = emit_front(0, outs)
            drain(fg, 10 ** 6)
            for it in range(nt):
                rg = emit_routing(it, *outs.pop(it))
                fg = emit_front(it + 1, outs) if it + 1 < nt else None
                r_live, f_live = True, True
                while r_live or f_live:
                    r_live = drain(rg, 3) if r_live else False
                    f_live = drain(fg, 1) if f_live else False

    nc.compile()
    return nc


_NC_CACHE: dict = {}


def _get_nc(nt: int) -> bass.Bass:
    if nt not in _NC_CACHE:
        _NC_CACHE[nt] = build_nc(nt)
    return _NC_CACHE[nt]


def make_in_maps(x, Wp, bp, W, nt: int = NT):
    """Shard + host-prep inputs for the SPMD launch (nt tiles per core)."""
    x = np.asarray(x, np.float32)
    cstb, cstf = build_consts(Wp, bp, W)
    bc = nt * NB
    maps = []
    for i in range(NCORES):
        xc = x[i * bc:(i + 1) * bc] if nt == NT else x[i * bc:(i + 1) * bc]
        maps.append({"xt": prep_x(xc), "cstb": cstb, "cstf": cstf})
    return maps


def gather_out(res, nt: int = NT):
    """[nt*80, NB] fp16 per core -> [ncores*nt*NB, C, CD] fp32."""
    parts = []
    for i in range(NCORES):
        vc = np.asarray(res.results[i]["vout"], np.float32)
        parts.append(vc.reshape(nt, C, CD, NB).transpose(0, 3, 1, 2)
                     .reshape(nt * NB, C, CD))
    return np.concatenate(parts, axis=0)


def kernel(x, Wp, bp, W):
    nc = _get_nc(NT)
    in_maps = make_in_maps(x, Wp, bp, W, NT)
    res = run_bass_kernel_spmd(nc, in_maps, list(range(NCORES)))
    return gather_out(res, NT).reshape(B, C, CD)
